# revision 1
# baseline (speedup 1.0000x reference)
"""GCN actor (2x GCNConv + linear heads) on 8 Trainium2 NeuronCores.

Strategy (dst-sharded graph parallel):
  - Nodes row-sharded 8 ways. Weights replicated. x arrives pre-transposed
    ([128, NSP], features on partitions) so there is no load-transpose pass.
  - Per layer: z = dinv * (h @ W) via TensorE from the persistent transposed
    activations hT; fp16 z rows are distributed into a replicated HBM table
    via chunk-aligned slice AllGathers (table in concat-of-slices order) that
    pipeline behind z_phase instead of forming one big barrier.
  - Aggregation per core over its in-edges (dst-owned, dst-sorted):
    transpose-mode dma_gather pulls source feature columns from the table,
    VectorE does exact segmented reduction over uniform-degree runs,
    partials are PE-transposed to token rows and dma_scatter_add'ed with
    the SBUF-destination mode (tokens_per_rank=128) into per-tile SBUF
    accumulators (token = tile-local node idx -> partition idx%128, parity
    of row-tile routes to accE/accO). No HBM accumulator, no zero-fill,
    no readback. Quota-pad partials land in a junk dump block.
  - finish per tile: h = relu(dinv*(acc+z)+b) in row space straight from
    the SBUF accumulator blocks (self-loop term dinv*z folded in), then
    PE-transposed into hT for the next layer / heads.
  - Gathers and scatters run on separate SWDGE queues (2 queues): their
    Q7 descriptor-generation streams process in parallel, which is worth
    ~1.4ms (the kernel is desc-processing bound at ~11-13 ns/index).
  - One Bass program serves all 8 cores (SPMD): the run schedule is padded
    to the max group-count over cores per (tile, chunk, degree-bucket);
    per-core variation lives in index/data inputs only.
"""

import os
import sys

for _p in ("/opt/trn_rl_repo", "/root/.axon_site/_ro/trn_rl_repo"):
    if os.path.isdir(_p) and _p not in sys.path:
        sys.path.insert(0, _p)

import numpy as np

import concourse.bacc as bacc
import concourse.bass as bass
import concourse.mybir as mybir
import concourse.tile as tile
from concourse.bass_utils import run_bass_kernel_spmd
from concourse.masks import make_identity

F = 128  # feature dim (fixed by problem)
NCORES = 8
CH = 32768  # rows addressable per gather call (int16 indices)
DCAP = 128  # max segment length per reduce group (never hit at avg deg 17)
DBUCKETS = list(range(1, DCAP + 1))
_BUCKET_LUT = np.zeros(DCAP + 1, dtype=np.int64)
for _d in range(1, DCAP + 1):
    _BUCKET_LUT[_d] = min(b for b in DBUCKETS if b >= _d)

ACC_BLKS = 9  # 8 real row-tile blocks per parity + 1 junk dump block
# SWDGE queues: gathers round-robin the first NQ-2 queues, scatters the rest;
# desc-gen streams on separate queues process in parallel
NQ = int(os.environ.get("KNQ", 2))

f32 = mybir.dt.float32
f16 = mybir.dt.float16
i16 = mybir.dt.int16


def _wrap16(flat, ncols):
    """Wrap a flat int16 index stream into the [128, ncols] layout the Q7
    gather/scatter ucode expects: slot i at [i % 16, i // 16], replicated
    across the eight 16-partition core groups."""
    n = flat.shape[0]
    assert n % 16 == 0 and n // 16 <= ncols
    a = np.full((16, ncols), -1, dtype=np.int16)
    a[:, : n // 16] = flat.reshape(n // 16, 16).T
    return np.tile(a, (8, 1))


class Schedule:
    """Uniform (cross-core) aggregation schedule + per-core index data."""

    def __init__(self, n_nodes, tile_nodes):
        self.N = n_nodes
        self.NS = n_nodes // NCORES
        self.NT = -(-self.NS // 128)  # node row-tiles per core
        self.NSP = self.NT * 128  # padded shard rows
        self.NROWS = self.NSP * NCORES  # gather table rows
        self.NCH = -(-self.NROWS // CH)  # chunks
        self.TSN = tile_nodes
        self.n_tiles = -(-self.NSP // tile_nodes)
        self.runs = None  # [t][k] -> list of (bucket, m)
        self.S = None  # [t][k] -> padded gather slots
        self.gcol0 = None  # [t][k] -> gidx col offset
        self.P = None  # [t][k] -> partials in chunk region (scatter num_idxs)
        self.scol0 = None  # [t][k] -> sidx col offset
        self.GCOLS = 0
        self.SCOLS = 0
        self.gidx = None  # per-core [128, GCOLS] int16
        self.sidx = None  # per-core [128, SCOLS] int16


def build_schedule(src, dst, n_nodes, tile_nodes=2048):
    """Host preprocessing: group edges by (dst-core, dst-node, src-chunk),
    bucket group sizes, take the max group count over cores per
    (tile, chunk, bucket) as the shared quota, and serialize per-core
    gather/scatter index streams. Tokens are parity-split tile-local."""
    sch = Schedule(n_nodes, tile_nodes)
    NS, NSP, NCH, TSN = sch.NS, sch.NSP, sch.NCH, sch.TSN
    n_tiles = sch.n_tiles

    src = np.asarray(src, dtype=np.int64)
    dst = np.asarray(dst, dtype=np.int64)
    src_row = (src // NS) * NSP + (src % NS)  # table row (padded layout)
    core = dst // NS
    nloc = dst % NS
    chunk = src_row // CH

    per_core = []
    for c in range(NCORES):
        m = core == c
        nl, ck, sr = nloc[m], chunk[m], src_row[m]
        order = np.lexsort((sr, ck, nl))
        nl, ck, sr = nl[order], ck[order], sr[order]
        newg = np.empty(nl.size, dtype=bool)
        newg[0] = True
        newg[1:] = (nl[1:] != nl[:-1]) | (ck[1:] != ck[:-1])
        starts = np.flatnonzero(newg)
        lens = np.diff(np.append(starts, nl.size))
        assert lens.max() <= DCAP, "degree cap exceeded"
        g_nl, g_ck = nl[starts], ck[starts]
        g_tile = g_nl // TSN
        g_bucket = _BUCKET_LUT[lens]
        per_core.append((g_nl, g_ck, lens, g_tile, g_bucket, sr, starts))

    # quotas: max #groups over cores per (tile, chunk, bucket)
    quota = {}
    for c in range(NCORES):
        g_nl, g_ck, lens, g_tile, g_bucket, sr, starts = per_core[c]
        key = (g_tile * NCH + g_ck) * (DCAP + 1) + g_bucket
        uk, cnt = np.unique(key, return_counts=True)
        for k, n in zip(uk, cnt):
            quota[int(k)] = max(quota.get(int(k), 0), int(n))

    runs = [[[] for _ in range(NCH)] for _ in range(n_tiles)]
    for k, n in sorted(quota.items()):
        b = k % (DCAP + 1)
        tk = k // (DCAP + 1)
        t, ck = tk // NCH, tk % NCH
        runs[t][ck].append((int(b), int(n)))
    for t in range(n_tiles):
        for ck in range(NCH):
            runs[t][ck].sort(key=lambda x: -x[0])  # big buckets first

    S = [[0] * NCH for _ in range(n_tiles)]
    gcol0 = [[0] * NCH for _ in range(n_tiles)]
    P = [[0] * NCH for _ in range(n_tiles)]
    scol0 = [[0] * NCH for _ in range(n_tiles)]
    gc = 0
    sc = 0
    for t in range(n_tiles):
        for ck in range(NCH):
            s = sum(b * m for b, m in runs[t][ck])
            s = -(-s // 128) * 128
            S[t][ck] = s
            gcol0[t][ck] = gc
            gc += s // 16
            p = sum(m for _, m in runs[t][ck])
            P[t][ck] = p
            scol0[t][ck] = sc
            sc += -(-p // 16)
    sch.runs, sch.S, sch.gcol0 = runs, S, gcol0
    sch.P, sch.scol0 = P, scol0
    sch.GCOLS, sch.SCOLS = max(gc, 16), max(sc, 16)

    # serialize per-core index streams
    sch.gidx, sch.sidx = [], []
    for c in range(NCORES):
        g_nl, g_ck, lens, g_tile, g_bucket, sr, starts = per_core[c]
        key = ((g_tile * NCH + g_ck) * (DCAP + 1) + g_bucket).astype(np.int64)
        order = np.argsort(key, kind="stable")
        k_sorted = key[order]
        gi = np.zeros((128, sch.GCOLS), dtype=np.int16)
        si = np.full((128, sch.SCOLS), -1, dtype=np.int16)
        for t in range(n_tiles):
            for ck in range(NCH):
                flat = []
                sflat = []
                for b, mq in runs[t][ck]:
                    kk = (t * NCH + ck) * (DCAP + 1) + b
                    lo = np.searchsorted(k_sorted, kk, "left")
                    hi = np.searchsorted(k_sorted, kk, "right")
                    mem = order[lo:hi]
                    mreal = hi - lo
                    assert mreal <= mq
                    mat = np.zeros((mq, b), dtype=np.int16)
                    if mreal:
                        l_g = lens[mem]
                        rows = np.repeat(np.arange(mreal), l_g)
                        cols = np.arange(l_g.sum()) - np.repeat(
                            np.cumsum(l_g) - l_g, l_g
                        )
                        take = np.repeat(starts[mem], l_g) + cols
                        mat[rows, cols] = (sr[take] - ck * CH).astype(np.int16)
                    flat.append(mat.reshape(-1))
                    # token = tile-local node idx; pads hit the dump slot
                    srow = np.full(mq, TSN, dtype=np.int16)
                    if mreal:
                        srow[:mreal] = (g_nl[mem] - t * TSN).astype(np.int16)
                    sflat.append(srow)
                flat = (
                    np.concatenate(flat) if flat else np.zeros(0, np.int16)
                )
                pad = S[t][ck] - flat.shape[0]
                flat = np.concatenate([flat, np.zeros(pad, np.int16)])
                nc_ = S[t][ck] // 16
                if nc_:
                    gi[:, gcol0[t][ck] : gcol0[t][ck] + nc_] = _wrap16(
                        flat, nc_
                    )
                sflat = (
                    np.concatenate(sflat) if sflat else np.zeros(0, np.int16)
                )
                ncs = -(-P[t][ck] // 16)
                pad = ncs * 16 - sflat.shape[0]
                sflat = np.concatenate([sflat, np.full(pad, -1, np.int16)])
                if ncs:
                    si[:, scol0[t][ck] : scol0[t][ck] + ncs] = _wrap16(
                        sflat, ncs
                    )
        sch.gidx.append(gi)
        sch.sidx.append(si)
    return sch


def build_bass(sch, repeat=1, fake_cc=False, ablate=()):
    """Build the single SPMD Bass program (see module docstring)."""
    NT, NSP, NROWS, NCH = sch.NT, sch.NSP, sch.NROWS, sch.NCH
    n_tiles = sch.n_tiles
    TSN = sch.TSN
    SMAX = max(max(r) for r in sch.S)
    RMAXB = max(-(-p // 128) for row in sch.P for p in row if p)
    SXMAX = max(-(-p // 16) for row in sch.P for p in row if p)

    nc = bacc.Bacc(
        "TRN2",
        target_bir_lowering=False,
        debug=False,
        enable_asserts=False,
        num_devices=1 if fake_cc else NCORES,
        num_swdge_queues=NQ,
        dynamic_dma_scratch_size=int(os.environ.get("KSCR", 16384)),
    )

    # I/O
    xT_in = nc.dram_tensor("xT", [128, NSP], f32, kind="ExternalInput").ap()
    gidx_in = nc.dram_tensor(
        "gidx", [128, sch.GCOLS], i16, kind="ExternalInput"
    ).ap()
    sidx_in = nc.dram_tensor(
        "sidx", [128, sch.SCOLS], i16, kind="ExternalInput"
    ).ap()
    dinv_in = nc.dram_tensor("dinv", [128, NT], f32, kind="ExternalInput").ap()
    w1_in = nc.dram_tensor("w1", [F, F], f32, kind="ExternalInput").ap()
    w2_in = nc.dram_tensor("w2", [F, F], f32, kind="ExternalInput").ap()
    wh_in = nc.dram_tensor("wh", [F, 32], f32, kind="ExternalInput").ap()
    b1_in = nc.dram_tensor("b1r", [128, F], f32, kind="ExternalInput").ap()
    b2_in = nc.dram_tensor("b2r", [128, F], f32, kind="ExternalInput").ap()
    bh_in = nc.dram_tensor("bhr", [128, 32], f32, kind="ExternalInput").ap()
    out_dram = nc.dram_tensor(
        "out", [NSP, 32], f32, kind="ExternalOutput"
    ).ap()

    # internal DRAM
    z_loc = [
        nc.dram_tensor(f"z_loc{i}", [NSP, F], f16).ap() for i in range(2)
    ]
    z_tab = [
        nc.dram_tensor(
            f"z_tab{i}",
            [NROWS, F],
            f16,
            addr_space="Local" if fake_cc else "Shared",
        ).ap()
        for i in range(2)
    ]

    rg = [list(range(NCORES))]

    with tile.TileContext(nc) as tc:
        with (
            tc.tile_pool(name="const", bufs=1) as constp,
            tc.tile_pool(name="big", bufs=1) as bigp,
            tc.tile_pool(name="msg", bufs=int(os.environ.get("KMSGB", 3))) as msgp,
            tc.tile_pool(
                name="arena", bufs=int(os.environ.get("KARB", 2))
            ) as arenap,
            tc.tile_pool(
                name="prow", bufs=int(os.environ.get("KPRB", 2))
            ) as prowp,
            tc.tile_pool(name="gix", bufs=int(os.environ.get("KGIXB", 4))) as gixp,
            tc.tile_pool(name="row", bufs=int(os.environ.get("KROWB", 2))) as rowp,
            tc.tile_pool(name="acc", bufs=int(os.environ.get("KACCB", 2))) as accp,
            tc.tile_pool(name="psum", bufs=3, space="PSUM") as psump,
            tc.tile_pool(name="psz", bufs=2, space="PSUM") as pszp,
        ):
            ident = constp.tile([128, 128], f32, tag="ident")
            make_identity(nc, ident[:])
            w1 = constp.tile([F, F], f32, tag="w1")
            nc.sync.dma_start(w1[:], w1_in)
            w2 = constp.tile([F, F], f32, tag="w2")
            nc.sync.dma_start(w2[:], w2_in)
            wh = constp.tile([F, 32], f32, tag="wh")
            nc.sync.dma_start(wh[:], wh_in)
            b1r = constp.tile([128, F], f32, tag="b1r")
            nc.sync.dma_start(b1r[:], b1_in)
            b2r = constp.tile([128, F], f32, tag="b2r")
            nc.sync.dma_start(b2r[:], b2_in)
            bhr = constp.tile([128, 32], f32, tag="bhr")
            nc.sync.dma_start(bhr[:], bh_in)
            dinv = constp.tile([128, NT], f32, tag="dinv")
            nc.sync.dma_start(dinv[:], dinv_in)

            # persistent transposed activations (x, then h1, h2)
            hT = bigp.tile([128, NSP], f32, tag="hT")
            for q in range(0, NT, 14):
                nb = min(14, NT - q)
                nc.sync.dma_start(
                    hT[:, q * 128 : (q + nb) * 128],
                    xT_in[:, q * 128 : (q + nb) * 128],
                )

            def z_phase(li, w):
                """z = dinv * (h @ W) from hT; write fp16 rows to z_loc,
                then AllGather into the replicated table."""
                zv = z_loc[li].rearrange("(a p) f -> p a f", p=128)
                for q in range(0, NT, 4):
                    nb = min(4, NT - q)
                    zr = rowp.tile([128, 4, F], f16, tag="zrow")
                    for j in range(nb):
                        ps = pszp.tile([128, F], f32, tag="zp")
                        nc.tensor.matmul(
                            ps[:],
                            lhsT=hT[:, (q + j) * 128 : (q + j + 1) * 128],
                            rhs=w[:],
                            start=True,
                            stop=True,
                        )
                        nc.scalar.activation(
                            zr[:, j, :],
                            ps[:],
                            mybir.ActivationFunctionType.Copy,
                            scale=dinv[:, q + j : q + j + 1],
                        )
                    nc.sync.dma_start(zv[:, q : q + nb, :], zr[:, :nb, :])
                if fake_cc or "cc" in ablate:
                    nc.sync.dma_start(z_tab[li][:NSP, :], z_loc[li][:, :])
                else:
                    nc.gpsimd.collective_compute(
                        "AllGather",
                        mybir.AluOpType.bypass,
                        replica_groups=rg,
                        ins=[z_loc[li][:, :]],
                        outs=[z_tab[li][:, :]],
                    )

            def finish_tile(li, brep, t, accE, accO):
                """h rows = relu(dinv*(acc + z) + b); transpose into hT.
                The self-loop term dinv^2*(hW) = dinv*z is folded in here."""
                tsn = min(TSN, NSP - t * TSN)
                nrt = tsn // 128  # row-tiles in this node-tile
                aE = accE.rearrange("p (a f) -> p a f", f=128)
                aO = accO.rearrange("p (a f) -> p a f", f=128)
                zv = z_loc[li].rearrange("(a p) f -> p a f", p=128)
                for r0 in range(0, nrt, 4):
                    nb = min(4, nrt - r0)
                    q0 = t * (TSN // 128) + r0
                    zt8 = rowp.tile([128, 4, F], f16, tag="zrd")
                    nc.sync.dma_start(zt8[:, :nb, :], zv[:, q0 : q0 + nb, :])
                    ps = psump.tile([128, 512], f32, tag="tph")
                    for i in range(nb):
                        r = r0 + i
                        q = q0 + i
                        at = (aE if r % 2 == 0 else aO)[:, r // 2, :]
                        nc.vector.tensor_tensor(
                            out=at, in0=at, in1=zt8[:, i, :],
                            op=mybir.AluOpType.add,
                        )
                        nc.vector.tensor_scalar_mul(
                            at, at, dinv[:, q : q + 1]
                        )
                        nc.vector.tensor_tensor(
                            out=at, in0=at, in1=brep[:],
                            op=mybir.AluOpType.add,
                        )
                        hr = rowp.tile([128, F], f32, tag="hrow")
                        nc.scalar.activation(
                            hr[:], at, mybir.ActivationFunctionType.Relu
                        )
                        nc.tensor.transpose(
                            ps[:, i * 128 : (i + 1) * 128], hr[:], ident[:]
                        )
                    nc.scalar.copy(
                        hT[:, q0 * 128 : (q0 + nb) * 128], ps[:, : nb * 128]
                    )

            def agg_phase(li, brep):
                for t in range(n_tiles):
                    accE = accp.tile([128, ACC_BLKS * 128], f32, tag="accE")
                    accO = accp.tile([128, ACC_BLKS * 128], f32, tag="accO")
                    nc.vector.memset(accE[:], 0.0)
                    nc.vector.memset(accO[:], 0.0)
                    prev_sc = None
                    for ck in range(NCH):
                        S = sch.S[t][ck]
                        P = sch.P[t][ck]
                        if S == 0:
                            continue
                        gx = gixp.tile([128, SMAX // 16], i16, tag="gx")
                        c0 = sch.gcol0[t][ck]
                        nc.sync.dma_start(
                            gx[:, : S // 16], gidx_in[:, c0 : c0 + S // 16]
                        )
                        msg = msgp.tile([128, SMAX], f16, tag="msg")
                        rows = min(CH, NROWS - ck * CH)
                        if "gather" in ablate:
                            continue
                        nc.gpsimd.dma_gather(
                            out_ap=msg[:, :S].rearrange(
                                "p (a s) -> p a s", a=1
                            ),
                            in_ap=z_tab[li][ck * CH : ck * CH + rows, :],
                            idxs_ap=gx[:, : S // 16],
                            num_idxs=S,
                            num_idxs_reg=S,
                            elem_size=F,
                            transpose=True,
                            single_packet=False,
                            queue_num=ck % max(NQ - 2, 1) if NQ > 2 else 0,
                        )
                        pblk = -(-P // 128)
                        arena = arenap.tile([128, RMAXB * 128], f32, tag="ar")
                        if "reduce" not in ablate:
                            off = 0
                            po = 0
                            for b, m in sch.runs[t][ck]:
                                nc.vector.tensor_reduce(
                                    out=arena[:, po : po + m],
                                    in_=msg[:, off : off + m * b].rearrange(
                                        "p (m b) -> p m b", b=b
                                    ),
                                    axis=mybir.AxisListType.X,
                                    op=mybir.AluOpType.add,
                                )
                                po += m
                                off += m * b
                        if "txp" in ablate:
                            continue
                        # transpose partials to token rows for the scatter;
                        # 4 transposes share one psum bank so one ACT copy
                        # moves [128, 512] per trip
                        pr = prowp.tile([128, RMAXB, 128], f32, tag="pr")
                        for bq in range(0, pblk, 4):
                            nb = min(4, pblk - bq)
                            ps = psump.tile([128, 512], f32, tag="tp")
                            for j in range(nb):
                                nc.tensor.transpose(
                                    ps[:, j * 128 : (j + 1) * 128],
                                    arena[
                                        :,
                                        (bq + j) * 128 : (bq + j + 1) * 128,
                                    ],
                                    ident[:],
                                )
                            nc.scalar.copy(
                                pr[:, bq : bq + nb, :], ps[:, : nb * 128]
                            )
                        if "scatter" in ablate:
                            continue
                        ncs = -(-P // 16)
                        sx = gixp.tile([128, SXMAX], i16, tag="sx")
                        s0 = sch.scol0[t][ck]
                        nc.sync.dma_start(
                            sx[:, :ncs], sidx_in[:, s0 : s0 + ncs]
                        )
                        sc_inst = nc.gpsimd.dma_scatter_add(
                            out_ap=accE[:, :],
                            in_ap=pr[:, :pblk, :],
                            idxs_ap=sx[:, :ncs],
                            num_idxs=P,
                            num_idxs_reg=P,
                            elem_size=F,
                            single_packet=False,
                            sbuf_tokens_per_rank=128,
                            parity_reg=0,
                            out_ap_other=accO[:, :],
                            queue_num=(NQ - 2 + (ck & 1)) if NQ > 2 else (NQ - 1),
                        )
                        if prev_sc is not None and "chain" not in ablate:
                            tile.add_dep_helper(
                                sc_inst.ins,
                                prev_sc.ins,
                                sync=True,
                                reason="serialize same-acc CCE RMW",
                            )
                        prev_sc = sc_inst
                    if "finish" not in ablate:
                        finish_tile(li, brep, t, accE, accO)

            def heads():
                ov = out_dram.rearrange("(a p) f -> p a f", p=128)
                for q in range(0, NT, 4):
                    nb = min(4, NT - q)
                    ot = rowp.tile([128, 4, 32], f32, tag="orow")
                    for j in range(nb):
                        ps = pszp.tile([128, 32], f32, tag="zp")
                        nc.tensor.matmul(
                            ps[:],
                            lhsT=hT[:, (q + j) * 128 : (q + j + 1) * 128],
                            rhs=wh[:],
                            start=True,
                            stop=True,
                        )
                        nc.vector.tensor_tensor(
                            out=ot[:, j, :], in0=ps[:], in1=bhr[:],
                            op=mybir.AluOpType.add,
                        )
                    nc.sync.dma_start(ov[:, q : q + nb, :], ot[:, :nb, :])

            for _rep in range(repeat):
                z_phase(0, w1)
                agg_phase(0, b1r)
                z_phase(1, w2)
                agg_phase(1, b2r)
                heads()

    nc.compile()
    return nc


def host_preprocess(inputs, n_nodes, tile_nodes=2048):
    x = np.asarray(inputs["x"], dtype=np.float32)
    ei = np.asarray(inputs["edge_index"])
    src, dst = ei[0].astype(np.int64), ei[1].astype(np.int64)

    deg = (np.bincount(dst, minlength=n_nodes) + 1).astype(np.float32)
    dinv = (1.0 / np.sqrt(deg)).astype(np.float32)

    sch = build_schedule(src, dst, n_nodes, tile_nodes)
    NS, NSP, NT = sch.NS, sch.NSP, sch.NT

    wh = np.concatenate(
        [np.asarray(inputs["Wm"], np.float32), np.asarray(inputs["Ws"], np.float32)],
        axis=1,
    )
    bh = np.concatenate(
        [np.asarray(inputs["bm"], np.float32), np.asarray(inputs["bs"], np.float32)]
    )
    b1 = np.asarray(inputs["b1"], np.float32)
    b2 = np.asarray(inputs["b2"], np.float32)

    in_maps = []
    for c in range(NCORES):
        xs = np.zeros((NSP, F), np.float32)
        xs[:NS] = x[c * NS : (c + 1) * NS]
        dv = np.ones(NSP, np.float32)
        dv[:NS] = dinv[c * NS : (c + 1) * NS]
        in_maps.append(
            {
                "xT": np.ascontiguousarray(xs.T),
                "gidx": sch.gidx[c],
                "sidx": sch.sidx[c],
                "dinv": dv.reshape(NT, 128).T.copy(),
                "w1": np.asarray(inputs["W1"], np.float32),
                "w2": np.asarray(inputs["W2"], np.float32),
                "wh": wh,
                "b1r": np.tile(b1[None, :], (128, 1)),
                "b2r": np.tile(b2[None, :], (128, 1)),
                "bhr": np.tile(bh[None, :], (128, 1)),
            }
        )
    return sch, in_maps


def run_gcn(inputs, n_nodes, tile_nodes=2048, trace=False, repeat=1, **run_kwargs):
    sch, in_maps = host_preprocess(inputs, n_nodes, tile_nodes)
    nc = build_bass(sch, repeat=repeat)
    res = run_bass_kernel_spmd(
        nc, in_maps, list(range(NCORES)), trace=trace, **run_kwargs
    )
    NS = sch.NS
    outs = [np.asarray(res.results[c]["out"])[:NS] for c in range(NCORES)]
    full = np.concatenate(outs, axis=0)
    mean = np.ascontiguousarray(full[:, :16])
    logstd = np.ascontiguousarray(full[:, 16:])
    return (mean, logstd), res


def kernel(**inputs):
    (mean, logstd), _ = run_gcn(inputs, n_nodes=100000)
    return mean, logstd



# revision 49
# speedup vs baseline: 48.8959x; 48.8959x over previous
"""GCN actor (2x GCNConv + linear heads) on 8 Trainium2 NeuronCores.

Strategy (dst-sharded graph parallel, mask-matmul aggregation):
  - Nodes row-sharded 8 ways. Weights replicated. x arrives pre-transposed
    ([128, NSP] f32, features on partitions) so there is no load-transpose.
  - Per layer: z = dinv * (h @ W) via TensorE from the persistent transposed
    activations hT; fp16 z rows are written per slab and AllGathered into a
    replicated HBM table whose row order is slab-major (slab s = concat of
    every core's local rows [SB[s], SB[s]+R[s]) ), so each AllGather slice is
    exactly one gather chunk (int16-addressable window).
  - Aggregation: per (slab, supertile-of-8-row-tiles) cell, a non-transpose
    dma_gather pulls each in-edge's source z row onto one partition (slot
    s -> partition s%128, block s//128). For every (block, dst-row-tile)
    pair the host schedules a matmul psum[feat,dst] += G_blk^T-style with a
    one-hot mask rhs built on VectorE in ONE batched is_equal (iota vs
    per-slot dst index, sentinel for pads / out-of-tile edges). Self-loops
    never enter the gather stream: each supertile adds its own z rows from
    the LOCAL z slab (one contiguous HWDGE DMA) through an identity-mask
    matmul. PSUM accumulates over a cell; a single VectorE add folds the
    cell into a [128, NSP] f32 SBUF accumulator.
    CRITICAL: the per-slot dst indices are bounced into PSUM (ACT copy)
    before the is_equal, so the mask build reads only ONE SBUF operand. A
    two-SBUF-input DVE op takes the shared SBUF port pair and locks GPSIMD
    out of writing SWDGE descriptors - the gather stream stalls behind it.
    This removes the scatter-add stream entirely; the only SWDGE traffic is
    the gather stream, load-balanced over all 4 SWDGE queues (each queue's
    descriptor generation runs on its own Q7 core pair, in parallel, at
    ~12 ns/index - the kernel's roofline).
  - finish per row-tile: hT[:, tile] = relu(acc * dinv_rows + b_col) with
    bias per-partition (features on partitions) - no transposes anywhere.
  - Cross-phase pipelining: the next layer's z rows + slab AllGathers are
    emitted inside the previous layer's final gather phase (per supertile),
    and heads interleave with layer-2 finishes, so collectives hide behind
    the gather stream.
  - One Bass program serves all 8 cores (SPMD): block->matmul schedule is
    the union over cores; per-core variation lives in index/mask data only.
"""

import os
import sys

for _p in ("/opt/trn_rl_repo", "/root/.axon_site/_ro/trn_rl_repo"):
    if os.path.isdir(_p) and _p not in sys.path:
        sys.path.insert(0, _p)

import numpy as np

import concourse.bacc as bacc
import concourse.bass as bass
import concourse.mybir as mybir
import concourse.tile as tile
from concourse.bass_utils import run_bass_kernel_spmd

F = 128
NCORES = 8
NS = 12500
NSP = 12544  # 98 row-tiles of 128
NT = 98
ST_TILES = 8  # row-tiles per supertile (psum region of 8*128 dst)
N_ST = 13  # ceil(98/8); last supertile has 2 tiles
R_SLAB = [3072, 3072, 3072, 3328]  # local rows per CC slab (tile-aligned)
SB = [0, 3072, 6144, 9216]  # local row base per slab
CHR = [8 * r for r in R_SLAB]  # gather-chunk rows (= AllGather slice rows)
NCH = 4
SENT = 8192.0  # f16-exact sentinel for "no dst" mask columns

NQ = 4  # SWDGE queues (max); all carry gathers round-robin
# 1: self-loops via local z slab + identity-mask matmul (no gather descs);
# 0: self-loops as ordinary gathered edges
KSELF = int(os.environ.get("KSELF", "1"))

f32 = mybir.dt.float32
f16 = mybir.dt.float16
i16 = mybir.dt.int16


def _wrap16(flat, ncols):
    """Wrap a flat int16 index stream into the [128, ncols] layout the Q7
    gather ucode expects: slot i at [i % 16, i // 16], replicated across the
    eight 16-partition core groups."""
    n = flat.shape[0]
    assert n % 16 == 0 and n // 16 <= ncols
    a = np.full((16, ncols), -1, dtype=np.int16)
    a[:, : n // 16] = flat.reshape(n // 16, 16).T
    return np.tile(a, (8, 1))


class Cell:
    __slots__ = ("s", "st", "B", "gcol0", "mm", "has_self", "queue")

    def __init__(self, s, st, B, gcol0, mm, has_self):
        self.s = s
        self.st = st
        self.B = B
        self.gcol0 = gcol0
        # mm: list of (block j, local tile l, start, stop, dr_col);
        # j == -1 is the self-loop block (local z slab + identity mask)
        self.mm = mm
        self.has_self = has_self
        self.queue = 0


class Schedule:
    pass


def build_schedule(src, dst):
    """Host preprocessing: group each core's in-edges (plus self-loops) by
    (src-slab, dst-supertile) cell, sorted by dst; take the max block count
    over cores as the cell quota; schedule matmuls for the union of dst
    row-tiles each block touches across cores; serialize per-core gather
    index streams (trailing -1 padded) and per-matmul dst-index columns."""
    sch = Schedule()
    src = np.asarray(src, dtype=np.int64)
    dst = np.asarray(dst, dtype=np.int64)
    sb = np.asarray(SB, dtype=np.int64)
    rs = np.asarray(R_SLAB, dtype=np.int64)

    cells_meta = [(s, st) for s in range(NCH) for st in range(N_ST)]
    n_cells = len(cells_meta)

    per_core = []
    for c in range(NCORES):
        m = dst // NS == c
        nl = (dst[m] % NS).astype(np.int64)
        q = (src[m] % NS).astype(np.int64)
        cs = src[m] // NS
        # self loops are normally NOT in the gather schedule: they are added
        # per supertile from the local z slab via a contiguous HWDGE DMA and
        # an identity-mask matmul (zero SWDGE descriptors).
        if not KSELF:
            loop = np.arange(NS, dtype=np.int64)
            nl = np.concatenate([nl, loop])
            q = np.concatenate([q, loop])
            cs = np.concatenate([cs, np.full(NS, c, dtype=np.int64)])
        s = np.minimum(q // 3072, 3)
        rel = cs * rs[s] + (q - sb[s])
        st = nl // (128 * ST_TILES)
        cell = s * N_ST + st
        order = np.lexsort((nl, cell))
        cell, nl, rel = cell[order], nl[order], rel[order]
        # cell boundaries
        bounds = np.searchsorted(cell, np.arange(n_cells + 1))
        per_core.append((bounds, nl, rel))

    cells = []
    gidx_parts = [[] for _ in range(NCORES)]
    dr_parts = [[] for _ in range(NCORES)]
    gcol = 0
    n_mm = 0
    slab_of_st_l = [0] * 3 + [1] * 3 + [2] * 3 + [3] * 4
    for ci, (s, st) in enumerate(cells_meta):
        Es = [
            per_core[c][0][ci + 1] - per_core[c][0][ci] for c in range(NCORES)
        ]
        B = max(-(-e // 128) for e in Es)
        has_self = KSELF and s == slab_of_st_l[st]
        if B == 0 and not has_self:
            continue
        nt_st = min(ST_TILES, NT - st * ST_TILES)
        # per-core padded dst-tile per slot (big sentinel for pads)
        slot_l = np.full((NCORES, B * 128), 10**6, dtype=np.int64)
        slot_nl = np.full((NCORES, B * 128), 10**9, dtype=np.int64)
        for c in range(NCORES):
            lo, hi = per_core[c][0][ci], per_core[c][0][ci + 1]
            nl_c = per_core[c][1][lo:hi]
            slot_nl[c, : Es[c]] = nl_c
            slot_l[c, : Es[c]] = nl_c // 128 - st * ST_TILES
        # union tile interval per block
        mm = []
        covered = set()
        for j in range(B):
            blk = slot_l[:, j * 128 : (j + 1) * 128]
            real = blk < 10**5
            if not real.any():
                continue
            lo_t = int(blk[real].min())
            hi_t = int(blk[real].max())
            for l in range(lo_t, hi_t + 1):
                mm.append([j, l, False, False, 0])
                covered.add(l)
        if has_self:
            # self-loop block per tile: lhsT from the local z slab (HWDGE
            # DMA, no gather descriptors), rhs = identity mask
            for l in range(nt_st):
                mm.append([-1, l, False, False, 0])
                covered.add(l)
        for l in range(nt_st):
            if l not in covered:
                mm.append([0, l, False, False, 0])  # dummy (all-sentinel)
                covered.add(l)
        # tile-major order: each tile's psum accumulation group opens and
        # closes before the next tile's (no interleaved start/stop groups)
        mm.sort(key=lambda e: (e[1], e[0]))
        # start/stop per local tile within this cell's psum
        first = {}
        last = {}
        for mi, (j, l, _, _, _) in enumerate(mm):
            if l not in first:
                first[l] = mi
            last[l] = mi
        for l, mi in first.items():
            mm[mi][2] = True
        for l, mi in last.items():
            mm[mi][3] = True
        # per-core data
        for c in range(NCORES):
            lo, hi = per_core[c][0][ci], per_core[c][0][ci + 1]
            rel_c = per_core[c][2][lo:hi]
            # pads duplicate a valid row (uniform num_idxs across cores);
            # their mask sentinel keeps them out of the accumulation
            fill = np.int16(rel_c[-1]) if Es[c] else np.int16(0)
            flat = np.full(B * 128, fill, dtype=np.int16)
            flat[: Es[c]] = rel_c.astype(np.int16)
            gidx_parts[c].append(_wrap16(flat, B * 8))
            # dstrel column per mm (self/dummy blocks keep the sentinel)
            cols = np.full((128, len(mm)), SENT, dtype=np.float16)
            for mi, (j, l, _, _, _) in enumerate(mm):
                if j < 0:
                    continue
                v = slot_nl[c, j * 128 : (j + 1) * 128] - (
                    st * ST_TILES + l
                ) * 128
                cols[:, mi] = np.where(
                    np.abs(v) < 4096, v, int(SENT)
                ).astype(np.float16)
            dr_parts[c].append(cols)
        for mi in range(len(mm)):
            mm[mi][4] = n_mm + mi
        cells.append(Cell(s, st, B, gcol, mm, has_self))
        gcol += B * 8
        n_mm += len(mm)

    # greedy per-queue descriptor load balancing
    qload = [0] * NQ
    for cell in cells:
        q = min(range(NQ), key=lambda i: qload[i])
        cell.queue = q
        qload[q] += cell.B * 128
    sch.qload = qload

    sch.cells = cells
    sch.GCOLS = max(gcol, 16)
    sch.MMTOT = n_mm
    sch.BMAX = max(c.B for c in cells)
    sch.MMCELLMAX = max(len(c.mm) for c in cells)
    sch.gidx = [
        np.concatenate(
            gidx_parts[c] + [np.full((128, sch.GCOLS - gcol), -1, np.int16)],
            axis=1,
        )
        for c in range(NCORES)
    ]
    sch.dstrel = [
        np.concatenate(dr_parts[c], axis=1).astype(np.float16)
        for c in range(NCORES)
    ]
    return sch


def build_bass(sch, repeat=1, fake_cc=False, ablate=()):
    ablate = set(ablate) | set(
        a for a in os.environ.get("KABL", "").split(",") if a
    )
    BMAX, MMCELLMAX = sch.BMAX, sch.MMCELLMAX
    nc = bacc.Bacc(
        "TRN2",
        target_bir_lowering=False,
        debug=False,
        enable_asserts=False,
        num_devices=1 if fake_cc else NCORES,
        num_swdge_queues=NQ,
        dynamic_dma_scratch_size=int(os.environ.get("KSCR", 16384)),
    )

    xT_in = nc.dram_tensor("xT", [128, NSP], f32, kind="ExternalInput").ap()
    gidx_in = nc.dram_tensor(
        "gidx", [128, sch.GCOLS], i16, kind="ExternalInput"
    ).ap()
    dr_in = nc.dram_tensor(
        "dstrel", [128, sch.MMTOT], f16, kind="ExternalInput"
    ).ap()
    dinv_in = nc.dram_tensor("dinv", [128, NT], f32, kind="ExternalInput").ap()
    dinvr_in = nc.dram_tensor(
        "dinvr", [128, NSP], f16, kind="ExternalInput"
    ).ap()
    iota_in = nc.dram_tensor(
        "iota", [128, 128], f32, kind="ExternalInput"
    ).ap()
    ident_in = nc.dram_tensor(
        "ident", [128, 128], f16, kind="ExternalInput"
    ).ap()
    w1_in = nc.dram_tensor("w1", [F, F], f32, kind="ExternalInput").ap()
    w2_in = nc.dram_tensor("w2", [F, F], f32, kind="ExternalInput").ap()
    wh_in = nc.dram_tensor("wh", [F, 32], f32, kind="ExternalInput").ap()
    b1_in = nc.dram_tensor("b1c", [128, 1], f32, kind="ExternalInput").ap()
    b2_in = nc.dram_tensor("b2c", [128, 1], f32, kind="ExternalInput").ap()
    bh_in = nc.dram_tensor("bhr", [128, 32], f32, kind="ExternalInput").ap()
    out_dram = nc.dram_tensor(
        "out", [NSP, 32], f32, kind="ExternalOutput"
    ).ap()

    z_loc = [
        [
            nc.dram_tensor(f"z_loc{i}_{s}", [R_SLAB[s], F], f16).ap()
            for s in range(NCH)
        ]
        for i in range(2)
    ]
    z_tab = [
        [
            nc.dram_tensor(
                f"z_tab{i}_{s}",
                [CHR[s], F],
                f16,
                addr_space="Local" if fake_cc else "Shared",
            ).ap()
            for s in range(NCH)
        ]
        for i in range(2)
    ]

    rg = [list(range(NCORES))]
    tile_slab = [0, 24, 48, 72, 98]  # row-tile bounds per slab

    with tile.TileContext(nc) as tc:
        with (
            tc.tile_pool(name="const", bufs=1) as constp,
            tc.tile_pool(name="big", bufs=1) as bigp,
            tc.tile_pool(name="row", bufs=3) as rowp,
            tc.tile_pool(name="dvr", bufs=2) as dvp,
            tc.tile_pool(name="g", bufs=int(os.environ.get("KGB", 4))) as gp,
            tc.tile_pool(name="gx", bufs=6) as gxp,
            tc.tile_pool(name="dr", bufs=4) as drp,
            tc.tile_pool(name="gs", bufs=2) as gsp,
            tc.tile_pool(name="mk", bufs=int(os.environ.get("KMKB", 3))) as mkp,
            tc.tile_pool(name="cps", bufs=2, space="PSUM") as cellp,
            tc.tile_pool(name="zps", bufs=2, space="PSUM") as pszp,
            tc.tile_pool(name="drps", bufs=2, space="PSUM") as drpsp,
        ):
            w1 = constp.tile([F, F], f32, tag="w1")
            nc.sync.dma_start(w1[:], w1_in)
            w2 = constp.tile([F, F], f32, tag="w2")
            nc.sync.dma_start(w2[:], w2_in)
            wh = constp.tile([F, 32], f32, tag="wh")
            nc.sync.dma_start(wh[:], wh_in)
            b1c = constp.tile([128, 1], f32, tag="b1c")
            nc.sync.dma_start(b1c[:], b1_in)
            b2c = constp.tile([128, 1], f32, tag="b2c")
            nc.sync.dma_start(b2c[:], b2_in)
            bhr = constp.tile([128, 32], f32, tag="bhr")
            nc.sync.dma_start(bhr[:], bh_in)
            dinv = constp.tile([128, NT], f32, tag="dinv")
            nc.sync.dma_start(dinv[:], dinv_in)
            iota = constp.tile([128, 128], f32, tag="iota")
            nc.sync.dma_start(iota[:], iota_in)
            ident = constp.tile([128, 128], f16, tag="ident")
            nc.sync.dma_start(ident[:], ident_in)

            hT = bigp.tile([128, NSP], f32, tag="hT")
            for q in range(0, NT, 14):
                nb = min(14, NT - q)
                nc.sync.dma_start(
                    hT[:, q * 128 : (q + nb) * 128],
                    xT_in[:, q * 128 : (q + nb) * 128],
                )
            acc = bigp.tile([128, NSP], f32, tag="acc")

            def z_rows(li, w, t0, t1):
                """z = dinv * (h @ W) rows for row-tiles [t0, t1) into the
                covering z_loc slab (slabs are supertile-aligned)."""
                s = 0
                while tile_slab[s + 1] <= t0:
                    s += 1
                assert t1 <= tile_slab[s + 1]
                zv = z_loc[li][s].rearrange("(a p) f -> p a f", p=128)
                for q in range(t0, t1, 4):
                    nb = min(4, t1 - q)
                    zr = rowp.tile([128, 4, F], f16, tag="zrow")
                    for j in range(nb):
                        ps = pszp.tile([128, F], f32, tag="zp")
                        nc.tensor.matmul(
                            ps[:],
                            lhsT=hT[:, (q + j) * 128 : (q + j + 1) * 128],
                            rhs=w[:],
                            start=True,
                            stop=True,
                        )
                        nc.scalar.activation(
                            zr[:, j, :],
                            ps[:],
                            mybir.ActivationFunctionType.Copy,
                            scale=dinv[:, q + j : q + j + 1],
                        )
                    a0 = q - tile_slab[s]
                    nc.sync.dma_start(zv[:, a0 : a0 + nb, :], zr[:, :nb, :])

            def z_cc(li, s):
                if fake_cc or "cc" in ablate:
                    nc.sync.dma_start(
                        z_tab[li][s][: R_SLAB[s], :], z_loc[li][s][:, :]
                    )
                else:
                    nc.gpsimd.collective_compute(
                        "AllGather",
                        mybir.AluOpType.bypass,
                        replica_groups=rg,
                        ins=[z_loc[li][s][:, :]],
                        outs=[z_tab[li][s][:, :]],
                    )

            def z_phase(li, w):
                for s in range(NCH):
                    z_rows(li, w, tile_slab[s], tile_slab[s + 1])
                    z_cc(li, s)

            def cell_mms(cell, G, mask, Gs):
                """Matmul-accumulate one cell's blocks into psum, then fold
                the cell into the SBUF accumulator."""
                if "pe" in ablate:
                    return
                ps = cellp.tile([128, ST_TILES * 128], f32, tag="cps")
                for mi, (j, l, st_f, sp_f, _) in enumerate(cell.mm):
                    nc.tensor.matmul(
                        ps[:, l * 128 : (l + 1) * 128],
                        lhsT=Gs[:, l, :] if j < 0 else G[:, j, :],
                        rhs=ident[:] if j < 0 else mask[:, mi, :],
                        start=st_f,
                        stop=sp_f,
                    )
                base = cell.st * ST_TILES * 128
                width = min(ST_TILES * 128, NSP - base)
                nc.vector.tensor_tensor(
                    out=acc[:, base : base + width],
                    in0=acc[:, base : base + width],
                    in1=ps[:, :width],
                    op=mybir.AluOpType.add,
                )

            def agg_phase(li, tail_cb=None):
                """Cells in slab-major order; after a supertile's final-slab
                cell folds into acc, run tail_cb(st) (finish + next-layer z /
                heads, overlapping the remaining gather stream)."""
                nc.vector.memset(acc[:], 0.0)
                staged = None  # one-cell lookahead: masks build ahead of adds
                todo = list(sch.cells) + [None]
                for cn, cell in enumerate(todo):
                    if cell is not None:
                        B = cell.B
                        nmm = len(cell.mm)
                        G = None
                        if B > 0:
                            gx = gxp.tile([128, BMAX * 8], i16, tag="gx")
                            nc.sync.dma_start(
                                gx[:, : B * 8],
                                gidx_in[:, cell.gcol0 : cell.gcol0 + B * 8],
                            )
                            G = gp.tile([128, BMAX, 128], f16, tag="G")
                            if "gather" not in ablate:
                                nc.gpsimd.dma_gather(
                                    out_ap=G[:, :B, :],
                                    in_ap=z_tab[li][cell.s][:, :],
                                    idxs_ap=gx[:, : B * 8],
                                    num_idxs=B * 128,
                                    num_idxs_reg=B * 128,
                                    elem_size=F,
                                    transpose=False,
                                    single_packet=False,
                                    queue_num=cell.queue,
                                )
                        Gs = None
                        if cell.has_self:
                            s = cell.s
                            a0 = cell.st * ST_TILES - tile_slab[s]
                            nt = min(ST_TILES, NT - cell.st * ST_TILES)
                            Gs = gsp.tile([128, ST_TILES, 128], f16, tag="Gs")
                            zv = z_loc[li][s].rearrange(
                                "(a p) f -> p a f", p=128
                            )
                            # scalar-engine HWDGE ring: keeps this bulk read
                            # out of the sync ring that feeds gather indices
                            nc.scalar.dma_start(
                                Gs[:, :nt, :], zv[:, a0 : a0 + nt, :]
                            )
                        if "mm" in ablate:
                            continue
                        dr = drp.tile([128, MMCELLMAX], f16, tag="dr")
                        c0 = cell.mm[0][4]
                        nc.sync.dma_start(
                            dr[:, :nmm], dr_in[:, c0 : c0 + nmm]
                        )
                        # bounce dr into PSUM via ACT so the mask build
                        # reads only ONE SBUF operand: a 2-SBUF-input DVE op
                        # takes the shared SBUF port pair and locks GPSIMD
                        # out of writing SWDGE descriptors (gather gen
                        # starves behind it)
                        drp_ps = drpsp.tile([128, 128], f32, tag="drp")
                        nc.scalar.copy(drp_ps[:, :nmm], dr[:, :nmm])
                        mask = mkp.tile(
                            [128, MMCELLMAX, 128], f16, tag="mask"
                        )
                        nc.vector.tensor_tensor(
                            out=mask[:, :nmm, :],
                            in0=iota[:][:, None, :].broadcast_to(
                                [128, nmm, 128]
                            ),
                            in1=drp_ps[:, :nmm, None].broadcast_to(
                                [128, nmm, 128]
                            ),
                            op=mybir.AluOpType.is_equal,
                        )
                    if staged is not None:
                        done = staged[0]
                        cell_mms(*staged)
                        if done.s == NCH - 1 and tail_cb is not None:
                            tail_cb(done.st)
                    staged = (
                        (cell, G, mask, Gs) if cell is not None else None
                    )
                if "mm" in ablate and tail_cb is not None:
                    for st in range(N_ST):
                        tail_cb(st)

            def finish_st(st, bcol):
                t0 = st * ST_TILES
                t1 = min(t0 + ST_TILES, NT)
                for q in range(t0, t1, 4):
                    nb = min(4, t1 - q)
                    dv = dvp.tile([128, 512], f16, tag="dv")
                    nc.sync.dma_start(
                        dv[:, : nb * 128],
                        dinvr_in[:, q * 128 : (q + nb) * 128],
                    )
                    for i in range(nb):
                        sl = slice((q + i) * 128, (q + i + 1) * 128)
                        nc.vector.tensor_tensor(
                            out=acc[:, sl],
                            in0=acc[:, sl],
                            in1=dv[:, i * 128 : (i + 1) * 128],
                            op=mybir.AluOpType.mult,
                        )
                        nc.scalar.activation(
                            hT[:, sl],
                            acc[:, sl],
                            mybir.ActivationFunctionType.Relu,
                            bias=bcol[:, 0:1],
                        )

            def heads_st(st):
                ov = out_dram.rearrange("(a p) f -> p a f", p=128)
                t0 = st * ST_TILES
                t1 = min(t0 + ST_TILES, NT)
                for q in range(t0, t1, 4):
                    nb = min(4, t1 - q)
                    ot = rowp.tile([128, 4, 32], f32, tag="orow")
                    for j in range(nb):
                        # shares the "zp" psum tag/shape to stay in budget
                        psz = pszp.tile([128, F], f32, tag="zp")
                        nc.tensor.matmul(
                            psz[:, :32],
                            lhsT=hT[:, (q + j) * 128 : (q + j + 1) * 128],
                            rhs=wh[:],
                            start=True,
                            stop=True,
                        )
                        nc.vector.tensor_tensor(
                            out=ot[:, j, :], in0=psz[:, :32], in1=bhr[:],
                            op=mybir.AluOpType.add,
                        )
                    nc.sync.dma_start(ov[:, q : q + nb, :], ot[:, :nb, :])

            # supertiles per z slab (slab boundaries are ST-aligned)
            slab_of_st = [0] * 3 + [1] * 3 + [2] * 3 + [3] * 4
            last_st_of_slab = {0: 2, 1: 5, 2: 8, 3: 12}

            def l1_tail(st):
                finish_st(st, b1c)
                t0 = st * ST_TILES
                t1 = min(t0 + ST_TILES, NT)
                z_rows(1, w2, t0, t1)
                s = slab_of_st[st]
                if last_st_of_slab[s] == st:
                    z_cc(1, s)

            def l2_tail(st, prefetch_next):
                finish_st(st, b2c)
                heads_st(st)
                if prefetch_next:
                    # pipeline the next body's layer-1 z/CC chain behind the
                    # remaining layer-2 gather stream
                    t0 = st * ST_TILES
                    t1 = min(t0 + ST_TILES, NT)
                    z_rows(0, w1, t0, t1)
                    s = slab_of_st[st]
                    if last_st_of_slab[s] == st:
                        z_cc(0, s)

            z_phase(0, w1)
            for _rep in range(repeat):
                nxt = _rep < repeat - 1
                agg_phase(0, tail_cb=l1_tail)
                agg_phase(
                    1, tail_cb=lambda st, n=nxt: l2_tail(st, n)
                )

    nc.compile()
    return nc


def host_preprocess(inputs, n_nodes=100000):
    x = np.asarray(inputs["x"], dtype=np.float32)
    ei = np.asarray(inputs["edge_index"])
    src, dst = ei[0].astype(np.int64), ei[1].astype(np.int64)

    deg = (np.bincount(dst, minlength=n_nodes) + 1).astype(np.float32)
    dinv = (1.0 / np.sqrt(deg)).astype(np.float32)

    sch = build_schedule(src, dst)

    wh = np.concatenate(
        [np.asarray(inputs["Wm"], np.float32),
         np.asarray(inputs["Ws"], np.float32)],
        axis=1,
    )
    bh = np.concatenate(
        [np.asarray(inputs["bm"], np.float32),
         np.asarray(inputs["bs"], np.float32)]
    )
    b1 = np.asarray(inputs["b1"], np.float32)
    b2 = np.asarray(inputs["b2"], np.float32)
    iota = np.tile(
        np.arange(128, dtype=np.float32)[None, :], (128, 1)
    )
    ident = np.eye(128, dtype=np.float16)

    in_maps = []
    for c in range(NCORES):
        xs = np.zeros((NSP, F), np.float32)
        xs[:NS] = x[c * NS : (c + 1) * NS]
        dvv = np.ones(NSP, np.float32)
        dvv[:NS] = dinv[c * NS : (c + 1) * NS]
        in_maps.append(
            {
                "xT": np.ascontiguousarray(xs.T),
                "gidx": sch.gidx[c],
                "dstrel": sch.dstrel[c],
                "dinv": dvv.reshape(NT, 128).T.copy(),
                "dinvr": np.tile(
                    dvv.astype(np.float16)[None, :], (128, 1)
                ),
                "iota": iota,
                "ident": ident,
                "w1": np.asarray(inputs["W1"], np.float32),
                "w2": np.asarray(inputs["W2"], np.float32),
                "wh": wh,
                "b1c": b1.reshape(128, 1).copy(),
                "b2c": b2.reshape(128, 1).copy(),
                "bhr": np.tile(bh[None, :], (128, 1)),
            }
        )
    return sch, in_maps


def run_gcn(inputs, n_nodes=100000, trace=False, repeat=1, **run_kwargs):
    sch, in_maps = host_preprocess(inputs, n_nodes)
    nc = build_bass(sch, repeat=repeat)
    res = run_bass_kernel_spmd(
        nc, in_maps, list(range(NCORES)), trace=trace, **run_kwargs
    )
    outs = [np.asarray(res.results[c]["out"])[:NS] for c in range(NCORES)]
    full = np.concatenate(outs, axis=0)
    mean = np.ascontiguousarray(full[:, :16])
    logstd = np.ascontiguousarray(full[:, 16:])
    return (mean, logstd), res


def kernel(**inputs):
    (mean, logstd), _ = run_gcn(inputs, n_nodes=100000)
    return mean, logstd


# revision 50
# speedup vs baseline: 55.8182x; 1.1416x over previous
"""GCN actor (2x GCNConv + linear heads) on 8 Trainium2 NeuronCores.

Strategy (dst-sharded graph parallel, mask-matmul aggregation):
  - Nodes row-sharded 8 ways. Weights replicated. x arrives pre-transposed
    ([128, NSP] f32, features on partitions) so there is no load-transpose.
  - Per layer: z = dinv * (h @ W) via TensorE from the persistent transposed
    activations hT; fp16 z rows are written per slab and AllGathered into a
    replicated HBM table whose row order is slab-major (slab s = concat of
    every core's local rows [SB[s], SB[s]+R[s]) ), so each AllGather slice is
    exactly one gather chunk (int16-addressable window).
  - Aggregation: per (slab, supertile-of-8-row-tiles) cell, a non-transpose
    dma_gather pulls each in-edge's source z row onto one partition (slot
    s -> partition s%128, block s//128). For every (block, dst-row-tile)
    pair the host schedules a matmul psum[feat,dst] += G_blk^T-style with a
    one-hot mask rhs built on VectorE in ONE batched is_equal (iota vs
    per-slot dst index, sentinel for pads / out-of-tile edges). Self-loops
    never enter the gather stream: each supertile adds its own z rows from
    the LOCAL z slab (one contiguous HWDGE DMA) through an identity-mask
    matmul. PSUM accumulates over a cell; a single VectorE add folds the
    cell into a [128, NSP] f32 SBUF accumulator.
    CRITICAL: the per-slot dst indices are bounced into PSUM (ACT copy)
    before the is_equal, so the mask build reads only ONE SBUF operand. A
    two-SBUF-input DVE op takes the shared SBUF port pair and locks GPSIMD
    out of writing SWDGE descriptors - the gather stream stalls behind it.
    This removes the scatter-add stream entirely; the only SWDGE traffic is
    the gather stream, load-balanced over all 4 SWDGE queues (each queue's
    descriptor generation runs on its own Q7 core pair, in parallel, at
    ~12 ns/index - the kernel's roofline).
  - finish per row-tile: hT[:, tile] = relu(acc * dinv_rows + b_col) with
    bias per-partition (features on partitions) - no transposes anywhere.
  - Cross-phase pipelining: the next layer's z rows + slab AllGathers are
    emitted inside the previous layer's final gather phase (per supertile),
    and heads interleave with layer-2 finishes, so collectives hide behind
    the gather stream.
  - One Bass program serves all 8 cores (SPMD): block->matmul schedule is
    the union over cores; per-core variation lives in index/mask data only.
"""

import os
import sys

for _p in ("/opt/trn_rl_repo", "/root/.axon_site/_ro/trn_rl_repo"):
    if os.path.isdir(_p) and _p not in sys.path:
        sys.path.insert(0, _p)

import numpy as np

import concourse.bacc as bacc
import concourse.bass as bass
import concourse.mybir as mybir
import concourse.tile as tile
from concourse.bass_utils import run_bass_kernel_spmd

F = 128
NCORES = 8
NS = 12500
NSP = 12544  # 98 row-tiles of 128
NT = 98
ST_TILES = 8  # row-tiles per supertile (psum region of 8*128 dst)
N_ST = 13  # ceil(98/8); last supertile has 2 tiles
R_SLAB = [3072, 3072, 3072, 3328]  # local rows per CC slab (tile-aligned)
SB = [0, 3072, 6144, 9216]  # local row base per slab
CHR = [8 * r for r in R_SLAB]  # gather-chunk rows (= AllGather slice rows)
NCH = 4
SENT = 8192.0  # f16-exact sentinel for "no dst" mask columns

NQ = 4  # SWDGE queues (max); all carry gathers round-robin
# 1: self-loops via local z slab + identity-mask matmul (no gather descs);
# 0: self-loops as ordinary gathered edges
KSELF = int(os.environ.get("KSELF", "1"))

f32 = mybir.dt.float32
f16 = mybir.dt.float16
i16 = mybir.dt.int16


def _wrap16(flat, ncols):
    """Wrap a flat int16 index stream into the [128, ncols] layout the Q7
    gather ucode expects: slot i at [i % 16, i // 16], replicated across the
    eight 16-partition core groups."""
    n = flat.shape[0]
    assert n % 16 == 0 and n // 16 <= ncols
    a = np.full((16, ncols), -1, dtype=np.int16)
    a[:, : n // 16] = flat.reshape(n // 16, 16).T
    return np.tile(a, (8, 1))


class Cell:
    __slots__ = ("s", "st", "B", "gcol0", "mm", "has_self", "queue")

    def __init__(self, s, st, B, gcol0, mm, has_self):
        self.s = s
        self.st = st
        self.B = B
        self.gcol0 = gcol0
        # mm: list of (block j, local tile l, start, stop, dr_col);
        # j == -1 is the self-loop block (local z slab + identity mask)
        self.mm = mm
        self.has_self = has_self
        self.queue = 0


class Schedule:
    pass


def build_schedule(src, dst):
    """Host preprocessing: group each core's in-edges by (src-slab,
    dst-supertile) cell, sorted by dst; take the max block count over cores
    as the cell quota; schedule matmuls for the union of dst row-tiles each
    block touches across cores; serialize per-core gather index streams
    (pads duplicate a valid row so num_idxs_reg stays uniform across cores;
    their sentinel mask columns zero the contribution) and per-matmul
    dst-index columns."""
    sch = Schedule()
    src = np.asarray(src, dtype=np.int64)
    dst = np.asarray(dst, dtype=np.int64)
    sb = np.asarray(SB, dtype=np.int64)
    rs = np.asarray(R_SLAB, dtype=np.int64)

    cells_meta = [(s, st) for s in range(NCH) for st in range(N_ST)]
    n_cells = len(cells_meta)

    per_core = []
    for c in range(NCORES):
        m = dst // NS == c
        nl = (dst[m] % NS).astype(np.int64)
        q = (src[m] % NS).astype(np.int64)
        cs = src[m] // NS
        # self loops are normally NOT in the gather schedule: they are added
        # per supertile from the local z slab via a contiguous HWDGE DMA and
        # an identity-mask matmul (zero SWDGE descriptors).
        if not KSELF:
            loop = np.arange(NS, dtype=np.int64)
            nl = np.concatenate([nl, loop])
            q = np.concatenate([q, loop])
            cs = np.concatenate([cs, np.full(NS, c, dtype=np.int64)])
        s = np.minimum(q // 3072, 3)
        rel = cs * rs[s] + (q - sb[s])
        st = nl // (128 * ST_TILES)
        cell = s * N_ST + st
        order = np.lexsort((nl, cell))
        cell, nl, rel = cell[order], nl[order], rel[order]
        # cell boundaries
        bounds = np.searchsorted(cell, np.arange(n_cells + 1))
        per_core.append((bounds, nl, rel))

    cells = []
    gidx_parts = [[] for _ in range(NCORES)]
    dr_parts = [[] for _ in range(NCORES)]
    gcol = 0
    n_mm = 0
    slab_of_st_l = [0] * 3 + [1] * 3 + [2] * 3 + [3] * 4
    for ci, (s, st) in enumerate(cells_meta):
        Es = [
            per_core[c][0][ci + 1] - per_core[c][0][ci] for c in range(NCORES)
        ]
        B = max(-(-e // 128) for e in Es)
        has_self = KSELF and s == slab_of_st_l[st]
        if B == 0 and not has_self:
            continue
        nt_st = min(ST_TILES, NT - st * ST_TILES)
        # per-core padded dst-tile per slot (big sentinel for pads)
        slot_l = np.full((NCORES, B * 128), 10**6, dtype=np.int64)
        slot_nl = np.full((NCORES, B * 128), 10**9, dtype=np.int64)
        for c in range(NCORES):
            lo, hi = per_core[c][0][ci], per_core[c][0][ci + 1]
            nl_c = per_core[c][1][lo:hi]
            slot_nl[c, : Es[c]] = nl_c
            slot_l[c, : Es[c]] = nl_c // 128 - st * ST_TILES
        # union tile interval per block
        mm = []
        covered = set()
        for j in range(B):
            blk = slot_l[:, j * 128 : (j + 1) * 128]
            real = blk < 10**5
            if not real.any():
                continue
            lo_t = int(blk[real].min())
            hi_t = int(blk[real].max())
            for l in range(lo_t, hi_t + 1):
                mm.append([j, l, False, False, 0])
                covered.add(l)
        if has_self:
            # self-loop block per tile: lhsT from the local z slab (HWDGE
            # DMA, no gather descriptors), rhs = identity mask
            for l in range(nt_st):
                mm.append([-1, l, False, False, 0])
                covered.add(l)
        for l in range(nt_st):
            if l not in covered:
                mm.append([0, l, False, False, 0])  # dummy (all-sentinel)
                covered.add(l)
        # tile-major order: each tile's psum accumulation group opens and
        # closes before the next tile's (no interleaved start/stop groups)
        mm.sort(key=lambda e: (e[1], e[0]))
        # start/stop per local tile within this cell's psum
        first = {}
        last = {}
        for mi, (j, l, _, _, _) in enumerate(mm):
            if l not in first:
                first[l] = mi
            last[l] = mi
        for l, mi in first.items():
            mm[mi][2] = True
        for l, mi in last.items():
            mm[mi][3] = True
        # per-core data
        for c in range(NCORES):
            lo, hi = per_core[c][0][ci], per_core[c][0][ci + 1]
            rel_c = per_core[c][2][lo:hi]
            # pads duplicate a valid row (uniform num_idxs across cores);
            # their mask sentinel keeps them out of the accumulation
            fill = np.int16(rel_c[-1]) if Es[c] else np.int16(0)
            flat = np.full(B * 128, fill, dtype=np.int16)
            flat[: Es[c]] = rel_c.astype(np.int16)
            gidx_parts[c].append(_wrap16(flat, B * 8))
            # dstrel column per mm (self/dummy blocks keep the sentinel)
            cols = np.full((128, len(mm)), SENT, dtype=np.float16)
            for mi, (j, l, _, _, _) in enumerate(mm):
                if j < 0:
                    continue
                v = slot_nl[c, j * 128 : (j + 1) * 128] - (
                    st * ST_TILES + l
                ) * 128
                cols[:, mi] = np.where(
                    np.abs(v) < 4096, v, int(SENT)
                ).astype(np.float16)
            dr_parts[c].append(cols)
        for mi in range(len(mm)):
            mm[mi][4] = n_mm + mi
        cells.append(Cell(s, st, B, gcol, mm, has_self))
        gcol += B * 8
        n_mm += len(mm)

    # greedy per-queue descriptor load balancing
    qload = [0] * NQ
    for cell in cells:
        q = min(range(NQ), key=lambda i: qload[i])
        cell.queue = q
        qload[q] += cell.B * 128
    sch.qload = qload

    sch.cells = cells
    sch.GCOLS = max(gcol, 16)
    sch.MMTOT = n_mm
    sch.BMAX = max(c.B for c in cells)
    sch.MMCELLMAX = max(len(c.mm) for c in cells)
    sch.gidx = [
        np.concatenate(
            gidx_parts[c] + [np.full((128, sch.GCOLS - gcol), -1, np.int16)],
            axis=1,
        )
        for c in range(NCORES)
    ]
    sch.dstrel = [
        np.concatenate(dr_parts[c], axis=1).astype(np.float16)
        for c in range(NCORES)
    ]
    return sch


def build_bass(sch, repeat=1, fake_cc=False, ablate=()):
    ablate = set(ablate) | set(
        a for a in os.environ.get("KABL", "").split(",") if a
    )
    BMAX, MMCELLMAX = sch.BMAX, sch.MMCELLMAX
    nc = bacc.Bacc(
        "TRN2",
        target_bir_lowering=False,
        debug=False,
        enable_asserts=False,
        num_devices=1 if fake_cc else NCORES,
        num_swdge_queues=NQ,
        dynamic_dma_scratch_size=int(os.environ.get("KSCR", 16384)),
    )

    xT_in = nc.dram_tensor("xT", [128, NSP], f32, kind="ExternalInput").ap()
    gidx_in = nc.dram_tensor(
        "gidx", [128, sch.GCOLS], i16, kind="ExternalInput"
    ).ap()
    dr_in = nc.dram_tensor(
        "dstrel", [128, sch.MMTOT], f16, kind="ExternalInput"
    ).ap()
    dinv_in = nc.dram_tensor("dinv", [128, NT], f32, kind="ExternalInput").ap()
    dinvr_in = nc.dram_tensor(
        "dinvr", [128, NSP], f16, kind="ExternalInput"
    ).ap()
    iota_in = nc.dram_tensor(
        "iota", [128, 128], f32, kind="ExternalInput"
    ).ap()
    ident_in = nc.dram_tensor(
        "ident", [128, 128], f16, kind="ExternalInput"
    ).ap()
    w1_in = nc.dram_tensor("w1", [F, F], f32, kind="ExternalInput").ap()
    w2_in = nc.dram_tensor("w2", [F, F], f32, kind="ExternalInput").ap()
    wh_in = nc.dram_tensor("wh", [F, 32], f32, kind="ExternalInput").ap()
    b1_in = nc.dram_tensor("b1c", [128, 1], f32, kind="ExternalInput").ap()
    b2_in = nc.dram_tensor("b2c", [128, 1], f32, kind="ExternalInput").ap()
    bh_in = nc.dram_tensor("bhr", [128, 32], f32, kind="ExternalInput").ap()
    out_dram = nc.dram_tensor(
        "out", [NSP, 32], f32, kind="ExternalOutput"
    ).ap()

    z_loc = [
        [
            nc.dram_tensor(f"z_loc{i}_{s}", [R_SLAB[s], F], f16).ap()
            for s in range(NCH)
        ]
        for i in range(2)
    ]
    z_tab = [
        [
            nc.dram_tensor(
                f"z_tab{i}_{s}",
                [CHR[s], F],
                f16,
                addr_space="Local" if fake_cc else "Shared",
            ).ap()
            for s in range(NCH)
        ]
        for i in range(2)
    ]

    rg = [list(range(NCORES))]
    tile_slab = [0, 24, 48, 72, 98]  # row-tile bounds per slab

    with tile.TileContext(nc) as tc:
        with (
            tc.tile_pool(name="const", bufs=1) as constp,
            tc.tile_pool(name="big", bufs=1) as bigp,
            tc.tile_pool(name="row", bufs=3) as rowp,
            tc.tile_pool(name="dvr", bufs=2) as dvp,
            tc.tile_pool(name="g", bufs=int(os.environ.get("KGB", 4))) as gp,
            tc.tile_pool(name="gx", bufs=6) as gxp,
            tc.tile_pool(name="dr", bufs=4) as drp,
            tc.tile_pool(name="gs", bufs=2) as gsp,
            tc.tile_pool(name="mk", bufs=int(os.environ.get("KMKB", 3))) as mkp,
            tc.tile_pool(name="cps", bufs=2, space="PSUM") as cellp,
            tc.tile_pool(name="zps", bufs=2, space="PSUM") as pszp,
            tc.tile_pool(name="drps", bufs=2, space="PSUM") as drpsp,
        ):
            w1 = constp.tile([F, F], f32, tag="w1")
            nc.sync.dma_start(w1[:], w1_in)
            w2 = constp.tile([F, F], f32, tag="w2")
            nc.sync.dma_start(w2[:], w2_in)
            wh = constp.tile([F, 32], f32, tag="wh")
            nc.sync.dma_start(wh[:], wh_in)
            b1c = constp.tile([128, 1], f32, tag="b1c")
            nc.sync.dma_start(b1c[:], b1_in)
            b2c = constp.tile([128, 1], f32, tag="b2c")
            nc.sync.dma_start(b2c[:], b2_in)
            bhr = constp.tile([128, 32], f32, tag="bhr")
            nc.sync.dma_start(bhr[:], bh_in)
            dinv = constp.tile([128, NT], f32, tag="dinv")
            nc.sync.dma_start(dinv[:], dinv_in)
            iota = constp.tile([128, 128], f32, tag="iota")
            nc.sync.dma_start(iota[:], iota_in)
            ident = constp.tile([128, 128], f16, tag="ident")
            nc.sync.dma_start(ident[:], ident_in)

            hT = bigp.tile([128, NSP], f32, tag="hT")
            for q in range(0, NT, 14):
                nb = min(14, NT - q)
                nc.sync.dma_start(
                    hT[:, q * 128 : (q + nb) * 128],
                    xT_in[:, q * 128 : (q + nb) * 128],
                )
            acc = bigp.tile([128, NSP], f32, tag="acc")

            def z_rows(li, w, t0, t1):
                """z = dinv * (h @ W) rows for row-tiles [t0, t1) into the
                covering z_loc slab (slabs are supertile-aligned)."""
                s = 0
                while tile_slab[s + 1] <= t0:
                    s += 1
                assert t1 <= tile_slab[s + 1]
                zv = z_loc[li][s].rearrange("(a p) f -> p a f", p=128)
                for q in range(t0, t1, 4):
                    nb = min(4, t1 - q)
                    zr = rowp.tile([128, 4, F], f16, tag="zrow")
                    for j in range(nb):
                        ps = pszp.tile([128, F], f32, tag="zp")
                        nc.tensor.matmul(
                            ps[:],
                            lhsT=hT[:, (q + j) * 128 : (q + j + 1) * 128],
                            rhs=w[:],
                            start=True,
                            stop=True,
                        )
                        nc.scalar.activation(
                            zr[:, j, :],
                            ps[:],
                            mybir.ActivationFunctionType.Copy,
                            scale=dinv[:, q + j : q + j + 1],
                        )
                    a0 = q - tile_slab[s]
                    nc.sync.dma_start(zv[:, a0 : a0 + nb, :], zr[:, :nb, :])

            def z_cc(li, s):
                if fake_cc or "cc" in ablate:
                    nc.sync.dma_start(
                        z_tab[li][s][: R_SLAB[s], :], z_loc[li][s][:, :]
                    )
                else:
                    nc.gpsimd.collective_compute(
                        "AllGather",
                        mybir.AluOpType.bypass,
                        replica_groups=rg,
                        ins=[z_loc[li][s][:, :]],
                        outs=[z_tab[li][s][:, :]],
                    )

            def z_phase(li, w):
                for s in range(NCH):
                    z_rows(li, w, tile_slab[s], tile_slab[s + 1])
                    z_cc(li, s)

            def cell_mms(cell, G, mask, Gs):
                """Matmul-accumulate one cell's blocks into psum, then fold
                the cell into the SBUF accumulator."""
                if "pe" in ablate:
                    return
                ps = cellp.tile([128, ST_TILES * 128], f32, tag="cps")
                for mi, (j, l, st_f, sp_f, _) in enumerate(cell.mm):
                    nc.tensor.matmul(
                        ps[:, l * 128 : (l + 1) * 128],
                        lhsT=Gs[:, l, :] if j < 0 else G[:, j, :],
                        rhs=ident[:] if j < 0 else mask[:, mi, :],
                        start=st_f,
                        stop=sp_f,
                    )
                base = cell.st * ST_TILES * 128
                width = min(ST_TILES * 128, NSP - base)
                nc.vector.tensor_tensor(
                    out=acc[:, base : base + width],
                    in0=acc[:, base : base + width],
                    in1=ps[:, :width],
                    op=mybir.AluOpType.add,
                )

            def agg_phase(li, tail_cb=None):
                """Cells in slab-major order; after a supertile's final-slab
                cell folds into acc, run tail_cb(st) (finish + next-layer z /
                heads, overlapping the remaining gather stream)."""
                nc.vector.memset(acc[:], 0.0)
                staged = None  # one-cell lookahead: masks build ahead of adds
                todo = list(sch.cells) + [None]
                for cn, cell in enumerate(todo):
                    if cell is not None:
                        B = cell.B
                        nmm = len(cell.mm)
                        G = None
                        if B > 0:
                            gx = gxp.tile([128, BMAX * 8], i16, tag="gx")
                            nc.sync.dma_start(
                                gx[:, : B * 8],
                                gidx_in[:, cell.gcol0 : cell.gcol0 + B * 8],
                            )
                            G = gp.tile([128, BMAX, 128], f16, tag="G")
                            if "gather" not in ablate:
                                nc.gpsimd.dma_gather(
                                    out_ap=G[:, :B, :],
                                    in_ap=z_tab[li][cell.s][:, :],
                                    idxs_ap=gx[:, : B * 8],
                                    num_idxs=B * 128,
                                    num_idxs_reg=B * 128,
                                    elem_size=F,
                                    transpose=False,
                                    single_packet=False,
                                    queue_num=cell.queue,
                                )
                        Gs = None
                        if cell.has_self:
                            s = cell.s
                            a0 = cell.st * ST_TILES - tile_slab[s]
                            nt = min(ST_TILES, NT - cell.st * ST_TILES)
                            Gs = gsp.tile([128, ST_TILES, 128], f16, tag="Gs")
                            zv = z_loc[li][s].rearrange(
                                "(a p) f -> p a f", p=128
                            )
                            # scalar-engine HWDGE ring: keeps this bulk read
                            # out of the sync ring that feeds gather indices
                            nc.scalar.dma_start(
                                Gs[:, :nt, :], zv[:, a0 : a0 + nt, :]
                            )
                        if "mm" in ablate:
                            continue
                        dr = drp.tile([128, MMCELLMAX], f16, tag="dr")
                        c0 = cell.mm[0][4]
                        nc.sync.dma_start(
                            dr[:, :nmm], dr_in[:, c0 : c0 + nmm]
                        )
                        # bounce dr into PSUM via ACT so the mask build
                        # reads only ONE SBUF operand: a 2-SBUF-input DVE op
                        # takes the shared SBUF port pair and locks GPSIMD
                        # out of writing SWDGE descriptors (gather gen
                        # starves behind it)
                        drp_ps = drpsp.tile([128, 128], f32, tag="drp")
                        nc.scalar.copy(drp_ps[:, :nmm], dr[:, :nmm])
                        mask = mkp.tile(
                            [128, MMCELLMAX, 128], f16, tag="mask"
                        )
                        nc.vector.tensor_tensor(
                            out=mask[:, :nmm, :],
                            in0=iota[:][:, None, :].broadcast_to(
                                [128, nmm, 128]
                            ),
                            in1=drp_ps[:, :nmm, None].broadcast_to(
                                [128, nmm, 128]
                            ),
                            op=mybir.AluOpType.is_equal,
                        )
                    if staged is not None:
                        done = staged[0]
                        cell_mms(*staged)
                        if done.s == NCH - 1 and tail_cb is not None:
                            tail_cb(done.st)
                    staged = (
                        (cell, G, mask, Gs) if cell is not None else None
                    )
                if "mm" in ablate and tail_cb is not None:
                    for st in range(N_ST):
                        tail_cb(st)

            def finish_st(st, bcol):
                t0 = st * ST_TILES
                t1 = min(t0 + ST_TILES, NT)
                for q in range(t0, t1, 4):
                    nb = min(4, t1 - q)
                    dv = dvp.tile([128, 512], f16, tag="dv")
                    nc.sync.dma_start(
                        dv[:, : nb * 128],
                        dinvr_in[:, q * 128 : (q + nb) * 128],
                    )
                    for i in range(nb):
                        sl = slice((q + i) * 128, (q + i + 1) * 128)
                        nc.vector.tensor_tensor(
                            out=acc[:, sl],
                            in0=acc[:, sl],
                            in1=dv[:, i * 128 : (i + 1) * 128],
                            op=mybir.AluOpType.mult,
                        )
                        nc.scalar.activation(
                            hT[:, sl],
                            acc[:, sl],
                            mybir.ActivationFunctionType.Relu,
                            bias=bcol[:, 0:1],
                        )

            def heads_st(st):
                ov = out_dram.rearrange("(a p) f -> p a f", p=128)
                t0 = st * ST_TILES
                t1 = min(t0 + ST_TILES, NT)
                for q in range(t0, t1, 4):
                    nb = min(4, t1 - q)
                    ot = rowp.tile([128, 4, 32], f32, tag="orow")
                    for j in range(nb):
                        # shares the "zp" psum tag/shape to stay in budget
                        psz = pszp.tile([128, F], f32, tag="zp")
                        nc.tensor.matmul(
                            psz[:, :32],
                            lhsT=hT[:, (q + j) * 128 : (q + j + 1) * 128],
                            rhs=wh[:],
                            start=True,
                            stop=True,
                        )
                        nc.vector.tensor_tensor(
                            out=ot[:, j, :], in0=psz[:, :32], in1=bhr[:],
                            op=mybir.AluOpType.add,
                        )
                    nc.sync.dma_start(ov[:, q : q + nb, :], ot[:, :nb, :])

            # supertiles per z slab (slab boundaries are ST-aligned)
            slab_of_st = [0] * 3 + [1] * 3 + [2] * 3 + [3] * 4
            last_st_of_slab = {0: 2, 1: 5, 2: 8, 3: 12}

            def l1_tail(st):
                finish_st(st, b1c)
                t0 = st * ST_TILES
                t1 = min(t0 + ST_TILES, NT)
                z_rows(1, w2, t0, t1)
                s = slab_of_st[st]
                if last_st_of_slab[s] == st:
                    z_cc(1, s)

            def l2_tail(st, prefetch_next):
                finish_st(st, b2c)
                heads_st(st)
                if prefetch_next:
                    # pipeline the next body's layer-1 z/CC chain behind the
                    # remaining layer-2 gather stream
                    t0 = st * ST_TILES
                    t1 = min(t0 + ST_TILES, NT)
                    z_rows(0, w1, t0, t1)
                    s = slab_of_st[st]
                    if last_st_of_slab[s] == st:
                        z_cc(0, s)

            z_phase(0, w1)
            for _rep in range(repeat):
                nxt = _rep < repeat - 1
                agg_phase(0, tail_cb=l1_tail)
                agg_phase(
                    1, tail_cb=lambda st, n=nxt: l2_tail(st, n)
                )

    nc.compile()
    return nc


def host_preprocess(inputs, n_nodes=100000):
    x = np.asarray(inputs["x"], dtype=np.float32)
    ei = np.asarray(inputs["edge_index"])
    src, dst = ei[0].astype(np.int64), ei[1].astype(np.int64)

    deg = (np.bincount(dst, minlength=n_nodes) + 1).astype(np.float32)
    dinv = (1.0 / np.sqrt(deg)).astype(np.float32)

    sch = build_schedule(src, dst)

    wh = np.concatenate(
        [np.asarray(inputs["Wm"], np.float32),
         np.asarray(inputs["Ws"], np.float32)],
        axis=1,
    )
    bh = np.concatenate(
        [np.asarray(inputs["bm"], np.float32),
         np.asarray(inputs["bs"], np.float32)]
    )
    b1 = np.asarray(inputs["b1"], np.float32)
    b2 = np.asarray(inputs["b2"], np.float32)
    iota = np.tile(
        np.arange(128, dtype=np.float32)[None, :], (128, 1)
    )
    ident = np.eye(128, dtype=np.float16)

    in_maps = []
    for c in range(NCORES):
        xs = np.zeros((NSP, F), np.float32)
        xs[:NS] = x[c * NS : (c + 1) * NS]
        dvv = np.ones(NSP, np.float32)
        dvv[:NS] = dinv[c * NS : (c + 1) * NS]
        in_maps.append(
            {
                "xT": np.ascontiguousarray(xs.T),
                "gidx": sch.gidx[c],
                "dstrel": sch.dstrel[c],
                "dinv": dvv.reshape(NT, 128).T.copy(),
                "dinvr": np.tile(
                    dvv.astype(np.float16)[None, :], (128, 1)
                ),
                "iota": iota,
                "ident": ident,
                "w1": np.asarray(inputs["W1"], np.float32),
                "w2": np.asarray(inputs["W2"], np.float32),
                "wh": wh,
                "b1c": b1.reshape(128, 1).copy(),
                "b2c": b2.reshape(128, 1).copy(),
                "bhr": np.tile(bh[None, :], (128, 1)),
            }
        )
    return sch, in_maps


def run_gcn(inputs, n_nodes=100000, trace=False, repeat=1, **run_kwargs):
    sch, in_maps = host_preprocess(inputs, n_nodes)
    nc = build_bass(sch, repeat=repeat)
    res = run_bass_kernel_spmd(
        nc, in_maps, list(range(NCORES)), trace=trace, **run_kwargs
    )
    outs = [np.asarray(res.results[c]["out"])[:NS] for c in range(NCORES)]
    full = np.concatenate(outs, axis=0)
    mean = np.ascontiguousarray(full[:, :16])
    logstd = np.ascontiguousarray(full[:, 16:])
    return (mean, logstd), res


def kernel(**inputs):
    (mean, logstd), _ = run_gcn(inputs, n_nodes=100000)
    return mean, logstd
